# revision 18
# baseline (speedup 1.0000x reference)
"""DigitCaps routing kernel for 8 Trainium2 NeuronCores.

Sharding: input_dim (1024 primary capsules) split 8 ways; per-core
preactivation partial sums are AllReduced each routing iteration.

Per core (I_c = 128 local capsules, handled as 64 pairs):
  phase 0: votes_i = inputs_i @ W_i  (fp32r matmuls, 2 capsules packed in the
           128x128 PE array via tile_position), evicted as bf16 to DRAM.
           A parallel accumulation matmul computes sum_i votes (uniform-route
           preactivation for routing iteration 0).
  pass k (k=1,2): stream votes back, compute agreement delta -> logits ->
           leaky softmax route -> route-weighted partial preactivation.
  Each iteration: AllReduce [64, 2048] partials, squash locally.
"""

import sys

if '/opt/trn_rl_repo' not in sys.path:
    sys.path.insert(0, '/opt/trn_rl_repo')

import numpy as np

import concourse.bacc as bacc
import concourse.mybir as mybir
import concourse.tile as tile
from concourse import masks
from concourse.bass_utils import run_bass_kernel_spmd

N_CORES = 8
B = 64          # batch
I_FULL = 1024   # primary capsules
C = 64          # input atoms
O = 64          # output capsules
A = 32          # output atoms
OA = O * A      # 2048
IC = I_FULL // N_CORES   # 128 local capsules
NPAIR = IC // 2          # 64
NGROUP = 8               # pairs per fp16 accumulation group

f32 = mybir.dt.float32
f32r = mybir.dt.float32r
bf16 = mybir.dt.bfloat16
f16 = mybir.dt.float16

LEAK_SCALE = 1.0 / (O + 1)  # route0 value: softmax of 65 zero logits


def _squash_factors(nc, pre, nsq, nrm, den, rec, fac, sq):
    """pre [P, OA] f32 -> fac [P, O] f32 (act = pre * fac), nsq/rec live out."""
    nc.vector.tensor_mul(sq[:], pre[:], pre[:])
    nc.vector.reduce_sum(
        nsq[:], sq[:].rearrange("p (o a) -> p o a", a=A), axis=mybir.AxisListType.X
    )
    nc.scalar.sqrt(nrm[:], nsq[:])
    nc.scalar.add(den[:], nsq[:], 1.0)
    nc.vector.reciprocal(rec[:], den[:])
    nc.vector.tensor_mul(fac[:], nrm[:], rec[:])


def _build(stage=99):
    nc = bacc.Bacc("TRN2", target_bir_lowering=False, debug=False,
                   num_devices=N_CORES)
    with tile.TileContext(nc) as tc:
        _emit(nc, tc, stage)
    nc.compile()
    return nc


def _emit(nc, tc, stage):
    x_d = nc.dram_tensor("inputs", [B, IC, C], f32, kind="ExternalInput")
    w_d = nc.dram_tensor("W", [IC, C, OA], f32, kind="ExternalInput")
    b_d = nc.dram_tensor("biases", [O, A], f32, kind="ExternalInput")
    out_d = nc.dram_tensor("out", [B, O], f32, kind="ExternalOutput")

    if True:
        with (
            tc.tile_pool(name="const", bufs=1) as cpool,
            tc.tile_pool(name="persist", bufs=1) as ppool,
            tc.tile_pool(name="dram", bufs=1, space="DRAM") as dpool,
        ):
            ident = cpool.tile([128, 128], f32)
            masks.make_identity(nc, ident[:])
            bias_sb = cpool.tile([128, OA], f32)
            nc.sync.dma_start(
                bias_sb[:],
                b_d[:].rearrange("o a -> (o a)").unsqueeze(0).broadcast_to((128, OA)),
            )

            inputsT = ppool.tile([128, NPAIR * B], bf16)  # [(c,pair-of-i), (p, b)]
            logits = ppool.tile([128, NPAIR * O], f32)    # [(b,parity), (p, o)]
            votes_dram = dpool.tile([NPAIR, 128, OA], bf16)
            ar_in = dpool.tile([B, OA], f32)
            ar_out = dpool.tile([B, OA], f32)

            # ---- phase A: load inputs, transpose pairs to [c, b] layout ----
            with (
                tc.tile_pool(name="xload", bufs=1) as xpool,
                tc.tile_pool(name="tpsum", bufs=2, space="PSUM") as tpsum,
            ):
                x_sb = xpool.tile([B, IC * C], f32)
                nc.sync.dma_start(
                    x_sb[:].rearrange("b (i c) -> b i c", c=C), x_d[:]
                )
                if stage <= 0:
                    dbg0 = ppool.tile([B, O], f32, tag="dbg")
                    nc.vector.tensor_add(dbg0[:], x_sb[:, 0:O], bias_sb[0:B, 0:O])
                    nc.sync.dma_start(out_d[:], dbg0[:])
                    return
                for p in range(NPAIR):
                    pt = tpsum.tile([128, B], f32)
                    nc.tensor.transpose(
                        pt[:], x_sb[:, p * 128:(p + 1) * 128], ident[0:B, 0:B]
                    )
                    nc.scalar.copy(inputsT[:, p * B:(p + 1) * B], pt[:])

            if stage <= 1:
                dbg = ppool.tile([B, O], f32, tag="dbg")
                nc.vector.tensor_copy(dbg[:], inputsT[0:B, 0:O].bitcast(bf16).bitcast(bf16))
                nc.sync.dma_start(out_d[:], dbg[:])
                return

            # ---- phase B: votes + uniform-route sum ----
            with (
                tc.tile_pool(name="wload", bufs=3) as wpool,
                tc.tile_pool(name="vpsum", bufs=2, space="PSUM") as vpsum,
                tc.tile_pool(name="accps", bufs=1, space="PSUM") as accpool,
                tc.tile_pool(name="vevict", bufs=4) as vepool,
            ):
                acc = accpool.tile([128, OA], f32)
                for p in range(NPAIR):
                    wt = wpool.tile([128, OA], f32, tag="wt")
                    nc.sync.dma_start(
                        wt[:],
                        w_d[2 * p:2 * p + 2].rearrange("t c oa -> (t c) oa"),
                    )
                    wt16 = wpool.tile([128, OA], bf16, tag="wt16")
                    nc.vector.tensor_copy(wt16[:], wt[:])
                    import os as _os
                    s2mode = _os.environ.get("S2MODE", "all")
                    lhs_pair = inputsT[:, p * B:(p + 1) * B]
                    if s2mode in ("all", "votes"):
                        for h in range(2):
                            vps = vpsum.tile([128, 1024], f32, tag="vps")
                            for q in range(2):
                                n0 = h * 1024 + q * 512
                                for io in range(2):
                                    pb = io * 64
                                    nc.tensor.matmul(
                                        vps[pb:pb + 64, q * 512:(q + 1) * 512],
                                        lhs_pair[pb:pb + 64, :],
                                        wt16[pb:pb + 64, n0:n0 + 512],
                                        start=True, stop=True,
                                        tile_position=(pb, pb),
                                    )
                            ve = vepool.tile([128, 1024], bf16, tag="ve")
                            nc.scalar.copy(ve[:], vps[:])
                            nc.sync.dma_start(
                                votes_dram[p][:, h * 1024:(h + 1) * 1024], ve[:]
                            )
                    # accumulation matmuls: sum over both capsules of the pair
                    par = p % 2
                    if s2mode in ("all", "acc"):
                        for q in range(4):
                            nc.tensor.matmul(
                                acc[par * 64:par * 64 + 64, q * 512:(q + 1) * 512],
                                lhs_pair,
                                wt16[:, q * 512:(q + 1) * 512],
                                start=(p == par), stop=(p >= NPAIR - 2),
                                tile_position=(0, par * 64),
                            )
                    elif p >= NPAIR - 2:
                        nc.tensor.matmul(
                            acc[par * 64:par * 64 + 64, 0:512],
                            lhs_pair, wt16[:, 0:512],
                            start=True, stop=True,
                            tile_position=(0, par * 64),
                        )
                # combine parities: partial0 = acc[0:64] + acc[64:128]
                accsb = ppool.tile([128, OA], f32, tag="accsb")
                nc.scalar.copy(accsb[:], acc[:])
                acc_hi = ppool.tile([B, OA], f32, tag="acc_hi")
                nc.sync.dma_start(acc_hi[:], accsb[B:128, :])
                partial0 = ppool.tile([B, OA], f32, tag="partial")
                nc.vector.tensor_add(partial0[:], accsb[0:B, :], acc_hi[:])
                nc.sync.dma_start(ar_in[:], partial0[:])

            if stage <= 2:
                nc.sync.dma_start(out_d[:], partial0[:, 0:O])
                return

            # ---- routing iterations ----
            act2 = ppool.tile([128, OA], bf16)        # act bcast to both halves
            wsum32 = ppool.tile([128, OA], f32)
            with (
                tc.tile_pool(name="vstream", bufs=3) as vspool,
                tc.tile_pool(name="passtmp", bufs=2) as tpool,
                tc.tile_pool(name="sqtmp", bufs=1) as qpool,
            ):
                s_full = qpool.tile([128, OA], f32, tag="s_full")
                pre = qpool.tile([128, OA], f32, tag="pre")
                sq = qpool.tile([128, OA], f32, tag="sq")
                nsq = qpool.tile([128, O], f32, tag="nsq")
                nrm = qpool.tile([128, O], f32, tag="nrm")
                den = qpool.tile([128, O], f32, tag="den")
                rec = qpool.tile([128, O], f32, tag="rec")
                fac = qpool.tile([128, O], f32, tag="fac")

                for it in range(3):
                    # AllReduce the preactivation partials
                    nc.gpsimd.collective_compute(
                        "AllReduce",
                        mybir.AluOpType.add,
                        replica_groups=[list(range(N_CORES))],
                        ins=[ar_in.opt()],
                        outs=[ar_out.opt()],
                    )
                    # broadcast-load to both partition halves
                    nc.sync.dma_start(
                        s_full[:],
                        ar_out[:].unsqueeze(0).broadcast_to((2, B, OA)),
                    )
                    scale = LEAK_SCALE if it == 0 else 1.0
                    # pre = s_full * scale + bias
                    nc.scalar.mul(pre[:], s_full[:], scale)
                    nc.vector.tensor_add(pre[:], pre[:], bias_sb[:])
                    _squash_factors(nc, pre, nsq, nrm, den, rec, fac, sq)
                    if stage <= 3:
                        nc.sync.dma_start(out_d[:], nsq[0:B, :])
                        return
                    if it == 2:
                        # out[b, o] = ||act|| = nsq * rec
                        final = qpool.tile([128, O], f32, tag="final")
                        nc.vector.tensor_mul(final[:], nsq[:], rec[:])
                        nc.sync.dma_start(out_d[:], final[0:B, :])
                        break
                    # act2 = pre * fac (broadcast over atoms), as bf16
                    nc.vector.tensor_tensor(
                        act2[:].rearrange("p (o a) -> p o a", a=A),
                        pre[:].rearrange("p (o a) -> p o a", a=A),
                        fac[:].unsqueeze(-1).broadcast_to((128, O, A)),
                        mybir.AluOpType.mult,
                    )

                    # streaming pass over votes
                    wacc = tpool.tile([128, OA], f16, tag="wacc")
                    for p in range(NPAIR):
                        vt = vspool.tile([128, OA], bf16, tag="vt")
                        nc.sync.dma_start(vt[:], votes_dram[p][:])
                        prod = tpool.tile([128, OA], bf16, tag="prod")
                        nc.vector.tensor_mul(prod[:], vt[:], act2[:])
                        delta = tpool.tile([128, O], f32, tag="delta")
                        nc.vector.reduce_sum(
                            delta[:],
                            prod[:].rearrange("p (o a) -> p o a", a=A),
                            axis=mybir.AxisListType.X,
                        )
                        lp = logits[:, p * O:(p + 1) * O]
                        if it == 0:
                            nc.vector.tensor_copy(lp, delta[:])
                        else:
                            nc.vector.tensor_add(lp, lp, delta[:])
                        expv = tpool.tile([128, O], f32, tag="expv")
                        esum = tpool.tile([128, 1], f32, tag="esum")
                        nc.scalar.activation(
                            expv[:], lp, mybir.ActivationFunctionType.Exp,
                            accum_out=esum[:],
                        )
                        edn = tpool.tile([128, 1], f32, tag="edn")
                        nc.scalar.add(edn[:], esum[:], 1.0)
                        erc = tpool.tile([128, 1], f32, tag="erc")
                        nc.vector.reciprocal(erc[:], edn[:])
                        route = tpool.tile([128, O], bf16, tag="route")
                        nc.vector.tensor_scalar_mul(route[:], expv[:], erc[:])
                        wv = tpool.tile([128, OA], f16, tag="wv")
                        nc.vector.tensor_tensor(
                            wv[:].rearrange("p (o a) -> p o a", a=A),
                            vt[:].rearrange("p (o a) -> p o a", a=A),
                            route[:].unsqueeze(-1).broadcast_to((128, O, A)),
                            mybir.AluOpType.mult,
                        )
                        g = p % NGROUP
                        if g == 0:
                            nc.vector.tensor_copy(wacc[:], wv[:])
                        else:
                            nc.vector.tensor_add(wacc[:], wacc[:], wv[:])
                        if g == NGROUP - 1:
                            if p == NGROUP - 1:
                                nc.vector.tensor_copy(wsum32[:], wacc[:])
                            else:
                                nc.vector.tensor_add(
                                    wsum32[:], wsum32[:], wacc[:]
                                )
                    # combine parity halves, ship to AllReduce
                    wsum_hi = ppool.tile([B, OA], f32, tag="acc_hi")
                    nc.sync.dma_start(wsum_hi[:], wsum32[B:128, :])
                    partial = ppool.tile([B, OA], f32, tag="partial")
                    nc.vector.tensor_add(
                        partial[:], wsum32[0:B, :], wsum_hi[:]
                    )
                    nc.sync.dma_start(ar_in[:], partial[:])
                    if stage <= 4:
                        nc.sync.dma_start(out_d[:], partial[:, 0:O])
                        return


_NC_CACHE = {}


def kernel(inputs, W, biases):
    if "nc" not in _NC_CACHE:
        _NC_CACHE["nc"] = _build()
    nc = _NC_CACHE["nc"]
    in_maps = []
    for k in range(N_CORES):
        sl = slice(k * IC, (k + 1) * IC)
        in_maps.append({
            "inputs": np.ascontiguousarray(inputs[:, sl, :], dtype=np.float32),
            "W": np.ascontiguousarray(W[sl], dtype=np.float32),
            "biases": np.ascontiguousarray(biases, dtype=np.float32),
        })
    res = run_bass_kernel_spmd(nc, in_maps, core_ids=list(range(N_CORES)))
    return res.results[0]["out"]


if __name__ == "__main__":
    rng = np.random.default_rng(0)
    inputs = rng.standard_normal((B, I_FULL, C)).astype(np.float32)
    W = (rng.standard_normal((I_FULL, C, OA)) * 0.02).astype(np.float32)
    biases = (rng.standard_normal((O, A)) * 0.01).astype(np.float32)
    out = kernel(inputs, W, biases)
    print("out shape:", out.shape, "sample:", out[0, :4])
